# revision 1
# baseline (speedup 1.0000x reference)
"""BiasAttention TRN2 kernel — q-sharded across 8 NeuronCores.

Each core owns a block of 128 queries and computes the full attention for
them (all 8 heads, all 1024 keys), including the z-bias projection, with no
collectives.  Host-side prep re-lays z out per core as [g, c, t, q] so the
bias-projection tiles arrive in SBUF matmul-ready (contract dim c on
partitions), and casts the matmul datapath to bf16 (TRN2 fp32 matmuls run
in split LOW/HIGH mode at ~4x the cost; accumulation stays fp32 in PSUM).
"""

import sys

if "/opt/trn_rl_repo" not in sys.path:
    sys.path.insert(0, "/opt/trn_rl_repo")

import ml_dtypes
import numpy as np

import concourse.bass as bass
import concourse.mybir as mybir
from concourse import bacc
from concourse.bass_utils import run_bass_kernel_spmd
from concourse.masks import make_identity
from concourse.tile import TileContext

P = 128          # partitions
H = 8            # heads
D = 32           # head dim
CQ = 256         # q channels
CKV = 256        # kv channels
BD = 128         # bias (z) channels
NQ = 1024        # total queries
NCORES = 8
NQC = NQ // NCORES   # queries per core = 128
SCALE = D ** (-0.5)

GK = 16          # z k-tiles per DMA group (shared host/device)
FP = mybir.dt.float32
BF = mybir.dt.bfloat16
NP_BF = ml_dtypes.bfloat16


def build_program(nk=1024, gk=GK):
    """Build the SPMD single-core program.  nk = number of keys, gk = z
    k-tiles per DMA group."""
    kc_n = nk // P            # k-chunks of 128
    ng = nk // gk             # z DMA groups
    add = mybir.AluOpType.add
    mult = mybir.AluOpType.mult

    nc = bacc.Bacc("TRN2", target_bir_lowering=False, debug=False,
                   num_devices=NCORES)

    # ---- I/O ----
    zT = nc.dram_tensor("zT", [ng, BD, gk, NQC], BF, kind="ExternalInput")
    xqT = nc.dram_tensor("xqT", [CQ, NQC], BF, kind="ExternalInput")
    xkvT = nc.dram_tensor("xkvT", [CKV, nk], BF, kind="ExternalInput")
    Wq = nc.dram_tensor("Wq", [CQ, H * D], BF, kind="ExternalInput")
    bq = nc.dram_tensor("bq", [H * D], FP, kind="ExternalInput")
    Wkv = nc.dram_tensor("Wkv", [CKV, 2 * H * D], BF, kind="ExternalInput")
    bkv = nc.dram_tensor("bkv", [2 * H * D], FP, kind="ExternalInput")
    Wb = nc.dram_tensor("Wb", [BD, H], BF, kind="ExternalInput")
    bb = nc.dram_tensor("bb", [H], FP, kind="ExternalInput")
    Wp = nc.dram_tensor("Wp", [H * D, CQ], FP, kind="ExternalInput")
    bp = nc.dram_tensor("bp", [CQ], FP, kind="ExternalInput")
    y = nc.dram_tensor("y", [NQC, CQ], FP, kind="ExternalOutput")

    with TileContext(nc) as tc:
        with (
            tc.tile_pool(name="const", bufs=1) as const,
            tc.tile_pool(name="zpool", bufs=24) as zpool,
            tc.tile_pool(name="epool", bufs=4) as epool,
            tc.tile_pool(name="atpool", bufs=4) as atpool,
            tc.tile_pool(name="proj_ps", bufs=2, space="PSUM") as proj_ps,
            tc.tile_pool(name="b_ps", bufs=3, space="PSUM") as b_psp,
            tc.tile_pool(name="t_ps", bufs=2, space="PSUM") as t_psp,
            tc.tile_pool(name="o_ps", bufs=1, space="PSUM") as o_psp,
        ):
            # ---- z stream head-start: wb + first z groups lead the ring ----
            wb_sb = const.tile([P, H], BF)
            nc.sync.dma_start(wb_sb, Wb[:])
            zpre = []
            for gidx in range(4):
                z_sb = zpool.tile([P, 16, NQC], BF, tag="z", name=f"zpre{gidx}")
                nc.sync.dma_start(z_sb, zT[gidx])
                zpre.append(z_sb)

            # ---- constants / weights to SBUF ----
            wq_sb = const.tile([P, 2, H * D], BF)
            nc.sync.dma_start(wq_sb, Wq.rearrange("(o p) m -> p o m", p=P))
            wkv_sb = const.tile([P, 2, 2 * H * D], BF)
            nc.sync.dma_start(wkv_sb, Wkv.rearrange("(o p) m -> p o m", p=P))
            wp_sb = const.tile([P, 2, CQ], FP)
            nc.sync.dma_start(wp_sb, Wp.rearrange("(o p) m -> p o m", p=P))
            xqT_sb = const.tile([P, 2, NQC], BF)
            nc.sync.dma_start(xqT_sb, xqT.rearrange("(o p) q -> p o q", p=P))
            xkvT_sb = const.tile([P, 2, nk], BF)
            nc.sync.dma_start(xkvT_sb, xkvT.rearrange("(o p) k -> p o k", p=P))
            bq_sb = const.tile([P, 2], FP)
            nc.sync.dma_start(bq_sb, bq.rearrange("(o p) -> p o", p=P))
            bkvK_sb = const.tile([P, 2], FP)
            nc.sync.dma_start(bkvK_sb, bkv[0:H * D].rearrange("(o p) -> p o", p=P))
            bkvV_sb = const.tile([1, H * D], FP)
            nc.sync.dma_start(bkvV_sb, bkv[None, H * D:2 * H * D])
            bp_sb = const.tile([1, CQ], FP)
            nc.sync.dma_start(bp_sb, bp[None, :])
            bb_ap = bb[:]
            bb_sb = const.tile([P, H], FP)
            nc.gpsimd.dma_start(
                out=bb_sb,
                in_=bass.AP(tensor=bb_ap.tensor, offset=bb_ap.offset,
                            ap=[[0, P]] + list(bb_ap.ap)),
            )
            ident = const.tile([P, P], FP)
            make_identity(nc, ident)
            ident_bf = const.tile([P, P], BF)
            make_identity(nc, ident_bf)
            ones_row = const.tile([1, P], FP)
            nc.vector.memset(ones_row, 1.0)

            # V augmented with a ones column per head: [k, kc, h, D+1]
            vaug_sb = const.tile([P, kc_n, H, D + 1], BF)
            nc.vector.memset(vaug_sb, 1.0)


            # ---- projections (bf16 in, fp32 psum accumulate) ----
            # Q^T [(h d), q] with (x + bq) * SCALE folded in, stored bf16
            qT_sb = const.tile([P, 2, NQC], BF)
            for m in range(2):
                ps = proj_ps.tile([P, 512], FP, tag="proj")
                for c in range(2):
                    nc.tensor.matmul(ps[:, :NQC],
                                     lhsT=wq_sb[:, c, m * P:(m + 1) * P],
                                     rhs=xqT_sb[:, c, :],
                                     start=(c == 0), stop=(c == 1))
                nc.vector.tensor_scalar(qT_sb[:, m, :], ps[:, :NQC],
                                        bq_sb[:, m:m + 1], SCALE, add, mult)

            # K^T [(h d), k] with +bkv_K, stored bf16
            kT_sb = const.tile([P, 2, nk], BF)
            for m in range(2):
                for nh in range((nk + 511) // 512):
                    nn_ = min(512, nk - nh * 512)
                    ps = proj_ps.tile([P, 512], FP, tag="proj")
                    for c in range(2):
                        nc.tensor.matmul(ps[:, :nn_],
                                         lhsT=wkv_sb[:, c, m * P:(m + 1) * P],
                                         rhs=xkvT_sb[:, c, nh * 512:nh * 512 + nn_],
                                         start=(c == 0), stop=(c == 1))
                    nc.vector.tensor_scalar(kT_sb[:, m, nh * 512:nh * 512 + nn_],
                                            ps[:, :nn_], bkvK_sb[:, m:m + 1],
                                            None, add)


            # S[q, h, k] = SCALE * Q K^T + bb  (scale folded into Q already)
            s_sb = const.tile([P, H, nk], FP)
            for h in range(H):
                r0 = (h % 4) * 32
                for nh in range((nk + 511) // 512):
                    nn_ = min(512, nk - nh * 512)
                    ps = proj_ps.tile([P, 512], FP, tag="proj", name="qk_ps")
                    nc.tensor.matmul(ps[:, :nn_],
                                     lhsT=qT_sb[r0:r0 + 32, h // 4, :],
                                     rhs=kT_sb[r0:r0 + 32, h // 4,
                                               nh * 512:nh * 512 + nn_],
                                     start=True, stop=True,
                                     tile_position=(r0, 0))
                    # Identity-with-bias folds bb[h] into S; alternate the
                    # copy between ACT and DVE to halve the serial chain.
                    if (h * 2 + nh) % 2 == 0:
                        nc.scalar.activation(
                            s_sb[:, h, nh * 512:nh * 512 + nn_], ps[:, :nn_],
                            mybir.ActivationFunctionType.Identity,
                            bias=bb_sb[:, h:h + 1])
                    else:
                        nc.vector.tensor_scalar(
                            s_sb[:, h, nh * 512:nh * 512 + nn_], ps[:, :nn_],
                            bb_sb[:, h:h + 1], None, add)



            # V [k, (h d)] + bkv_V, written into vaug (ones col preserved);
            # emitted after QK so the S chain starts first.
            for kc in range(kc_n):
                ps = proj_ps.tile([P, 512], FP, tag="proj", name="v_ps")
                for c in range(2):
                    nc.tensor.matmul(ps[:, :H * D],
                                     lhsT=xkvT_sb[:, c, kc * P:(kc + 1) * P],
                                     rhs=wkv_sb[:, c, H * D:2 * H * D],
                                     start=(c == 0), stop=False)
                nc.tensor.matmul(ps[:, :H * D], lhsT=ones_row,
                                 rhs=bkvV_sb, start=False, stop=True)
                nc.scalar.activation(
                    vaug_sb[:, kc, :, 0:D],
                    ps[:, :H * D].rearrange("p (h d) -> p h d", h=H),
                    mybir.ActivationFunctionType.Copy)
            # ---- main loop over k-chunks ----
            o_ps = o_psp.tile([P, H * (D + 1)], FP)   # [q, h*(D+1)]
            gpc = P // gk if gk < P else 1      # groups per k-chunk
            tpg = min(gk, P)                    # k-tiles per group
            HKT = 64                      # k-tiles per half-chunk (1 psum bank)
            gph = HKT // tpg              # z DMA groups per half-chunk
            for kc in range(kc_n):
                x_sb = epool.tile([P, H, P], BF, tag="x")
                for hf in range(2):
                    # z-bias matmuls accumulate into b_ps [q, kt*H + h]
                    b_ps = b_psp.tile([P, HKT * H], FP, tag="b")
                    for g in range(gph):
                        gidx = kc * gpc + hf * gph + g
                        if gidx < len(zpre):
                            z_sb = zpre[gidx]
                        else:
                            z_sb = zpool.tile([P, tpg, NQC], BF, tag="z")
                            nc.sync.dma_start(z_sb, zT[gidx])
                        for t in range(tpg):
                            kt = g * tpg + t
                            nc.tensor.matmul(b_ps[:, kt * H:(kt + 1) * H],
                                             lhsT=z_sb[:, t, :], rhs=wb_sb,
                                             start=(kt == 0),
                                             stop=(kt == HKT - 1))
                    # batched add + exp for this half-chunk (all 8 heads)
                    e_sb = epool.tile([P, H, HKT], FP, tag="e")
                    nc.vector.tensor_tensor(
                        e_sb,
                        s_sb[:, :, kc * P + hf * HKT:kc * P + (hf + 1) * HKT],
                        b_ps.rearrange("p (kt h) -> p h kt", h=H), add)
                    nc.scalar.activation(x_sb[:, :, hf * HKT:(hf + 1) * HKT],
                                         e_sb,
                                         mybir.ActivationFunctionType.Exp)
                for hg in range(2):          # head groups of 4
                    t_ps = t_psp.tile([P, 4, P], BF, tag="t")
                    for hl in range(4):
                        nc.tensor.transpose(t_ps[:, hl, :],
                                            x_sb[:, hg * 4 + hl, :], ident_bf)
                    at_sb = atpool.tile([P, 4, P], BF, tag="at")
                    nc.vector.tensor_copy(at_sb, t_ps)
                    for hl in range(4):
                        h = hg * 4 + hl
                        # o_ps lives in one bank: open the accumulation group
                        # on the first matmul only, close on the last.
                        nc.tensor.matmul(
                            o_ps[:, h * (D + 1):(h + 1) * (D + 1)],
                            lhsT=at_sb[:, hl, :], rhs=vaug_sb[:, kc, h, :],
                            start=(kc == 0 and h == 0),
                            stop=(kc == kc_n - 1 and h == H - 1))

            # ---- epilogue: normalize, transpose, output projection ----
            recip_sb = const.tile([P, H], FP)
            for h in range(H):
                nc.vector.reciprocal(recip_sb[:, h:h + 1],
                                     o_ps[:, h * (D + 1) + D:h * (D + 1) + D + 1])
            o_sb = const.tile([P, 2, P], FP)     # [q, half, (h d)%128]
            for h in range(H):
                nc.vector.tensor_scalar(
                    o_sb[:, h // 4, (h % 4) * 32:(h % 4) * 32 + 32],
                    o_ps[:, h * (D + 1):h * (D + 1) + D],
                    recip_sb[:, h:h + 1], None, mult)
            oT_sb = const.tile([P, 2, P], FP)
            for m in range(2):
                t_full = proj_ps.tile([P, 512], FP, tag="proj", name="t_full")
                t_ps = t_full[:, :P]
                nc.tensor.transpose(t_ps, o_sb[:, m, :], ident)
                nc.vector.tensor_copy(oT_sb[:, m, :], t_ps)
            ps = proj_ps.tile([P, 512], FP, tag="proj")
            for m in range(2):
                nc.tensor.matmul(ps[:, :CQ], lhsT=oT_sb[:, m, :],
                                 rhs=wp_sb[:, m, :], start=(m == 0), stop=False)
            nc.tensor.matmul(ps[:, :CQ], lhsT=ones_row, rhs=bp_sb,
                             start=False, stop=True)
            y_sb = const.tile([P, CQ], FP)
            nc.vector.tensor_copy(y_sb, ps[:, :CQ])
            nc.sync.dma_start(y[:], y_sb)

    nc.compile()
    return nc


def prep_inputs(x_q, x_kv, z, Wq, bq, Wkv, bkv, Wb, bb, Wp, bp,
                nk=1024, gk=GK):
    """Host-side shard prep.  Returns in_maps for the 8 cores."""
    ng = nk // gk
    xkvT = np.ascontiguousarray(x_kv[0].T).astype(NP_BF)     # [CKV, nk]
    shared = dict(xkvT=xkvT,
                  Wq=np.ascontiguousarray(Wq).astype(NP_BF),
                  bq=np.ascontiguousarray(bq, dtype=np.float32),
                  Wkv=np.ascontiguousarray(Wkv).astype(NP_BF),
                  bkv=np.ascontiguousarray(bkv, dtype=np.float32),
                  Wb=np.ascontiguousarray(Wb).astype(NP_BF),
                  bb=np.ascontiguousarray(bb, dtype=np.float32),
                  Wp=np.ascontiguousarray(Wp, dtype=np.float32),
                  bp=np.ascontiguousarray(bp, dtype=np.float32))
    in_maps = []
    for i in range(NCORES):
        qs = i * NQC
        zi = z[0, qs:qs + NQC]                           # [q, k, c]
        # -> [g, c, t, q] with k = g*gk + t
        zi = zi.reshape(NQC, ng, gk, BD).transpose(1, 3, 2, 0)
        in_maps.append(dict(
            zT=np.ascontiguousarray(zi).astype(NP_BF),
            xqT=np.ascontiguousarray(x_q[0, qs:qs + NQC].T).astype(NP_BF),
            **shared,
        ))
    return in_maps


_NC_CACHE = {}


def kernel(x_q, x_kv, z, Wq, bq, Wkv, bkv, Wb, bb, Wp, bp):
    key = "full"
    if key not in _NC_CACHE:
        _NC_CACHE[key] = build_program()
    nc = _NC_CACHE[key]
    in_maps = prep_inputs(x_q, x_kv, z, Wq, bq, Wkv, bkv, Wb, bb, Wp, bp)
    res = run_bass_kernel_spmd(nc, in_maps, list(range(NCORES)))
    out = np.empty((1, NQ, CQ), dtype=np.float32)
    for i in range(NCORES):
        out[0, i * NQC:(i + 1) * NQC, :] = res.results[i]["y"]
    return out



# revision 17
# speedup vs baseline: 1.1377x; 1.1377x over previous
"""BiasAttention TRN2 kernel — q-sharded across 8 NeuronCores, fp8 z.

Each core owns 128 queries and computes full attention for them (8 heads,
1024 keys) with no collectives.  The dominant cost is streaming the bias
tensor z ([q,k,c] = 16.8 MB/core in fp8): host prep casts z to e3m4
(4 mantissa bits; z~N(0,1) fits the +-15.5 range) and lays it out
[kc*2+half, c, q, k] so each [c=128, k=128] slice is LDWEIGHTS-ready.

Scores are computed directly in the transposed frame S^T[k, q] (lhsT = K^T)
so the exp output feeds the AV matmul without any PE transposes.  Q^T/K^T
are stored per-head on partitions 0-31 ([32, h, n]) so score matmuls never
need tile_position (multi-matmul PSUM accumulation with row-positioned
strips aborts at load on this runtime).  Per k-chunk: two score banks
[k, (4h, 128q)], and two bias banks [k, (64q, 8h)] accumulating a
ones x bb row plus 64 z matmuls (lhsT = z[c,k] per query, rhs =
e3m4(64*Wb); fp8 weight loads take the fast-weight-load path).  The 1/64
Wb scale (needed to lift Wb out of e3m4's subnormal range) is folded into
the exp activations via exp(a+b) = exp(a)*exp(b): each PSUM bank gets its
own exp (ACT reads one PSUM operand), and a DVE multiply fuses them into
bf16 slabs that are the AV lhsT as-is.  A ones column appended to V
accumulates the softmax denominator in the same AV pass.
"""

import sys

if "/opt/trn_rl_repo" not in sys.path:
    sys.path.insert(0, "/opt/trn_rl_repo")

import ml_dtypes
import numpy as np

import concourse.bass as bass
import concourse.mybir as mybir
from concourse import bacc
from concourse.bass_utils import run_bass_kernel_spmd
from concourse.masks import make_identity
from concourse.tile import TileContext

P = 128          # partitions
H = 8            # heads
D = 32           # head dim
CQ = 256         # q channels
CKV = 256        # kv channels
BD = 128         # bias (z) channels
NQ = 1024        # total queries
NK = 1024        # total keys
NCORES = 8
NQC = NQ // NCORES   # queries per core = 128
KC_N = NK // P       # k-chunks of 128
QH = NQC // 2        # queries per bias psum bank = 64
SCALE = D ** (-0.5)
WBS = 64.0           # Wb host-side scale (power of two)

FP = mybir.dt.float32
BF = mybir.dt.bfloat16
F8 = mybir.dt.float8e3
NP_BF = ml_dtypes.bfloat16
NP_F8 = ml_dtypes.float8_e3m4


def build_program():
    add = mybir.AluOpType.add
    mult = mybir.AluOpType.mult

    nc = bacc.Bacc("TRN2", target_bir_lowering=False, debug=False,
                   num_devices=NCORES)

    # ---- I/O ----
    zT = nc.dram_tensor("zT", [2 * KC_N, BD, QH, P], F8, kind="ExternalInput")
    xqT = nc.dram_tensor("xqT", [CQ, NQC], BF, kind="ExternalInput")
    xkvT = nc.dram_tensor("xkvT", [CKV, NK], BF, kind="ExternalInput")
    Wq = nc.dram_tensor("Wq", [CQ, H * D], BF, kind="ExternalInput")
    bq2 = nc.dram_tensor("bq2", [D, H], FP, kind="ExternalInput")
    Wkv = nc.dram_tensor("Wkv", [CKV, 2 * H * D], BF, kind="ExternalInput")
    bkvK2 = nc.dram_tensor("bkvK2", [D, H], FP, kind="ExternalInput")
    bkvV = nc.dram_tensor("bkvV", [H * D], BF, kind="ExternalInput")
    Wb64 = nc.dram_tensor("Wb64", [BD, H], F8, kind="ExternalInput")
    bbpat = nc.dram_tensor("bbpat", [H * QH], BF, kind="ExternalInput")
    Wp = nc.dram_tensor("Wp", [H * D, CQ], FP, kind="ExternalInput")
    bp = nc.dram_tensor("bp", [CQ], FP, kind="ExternalInput")
    y = nc.dram_tensor("y", [NQC, CQ], FP, kind="ExternalOutput")

    with TileContext(nc) as tc:
        with (
            tc.tile_pool(name="const", bufs=1) as const,
            tc.tile_pool(name="zpool", bufs=6) as zpool,
            tc.tile_pool(name="epool", bufs=2) as epool,
            tc.tile_pool(name="proj_ps", bufs=2, space="PSUM") as proj_ps,
            tc.tile_pool(name="b_ps", bufs=2, space="PSUM") as b_psp,
            tc.tile_pool(name="s_ps", bufs=2, space="PSUM") as s_psp,
            tc.tile_pool(name="o_ps", bufs=1, space="PSUM") as o_psp,
        ):
            # ---- small weights first (prologue deps), then the z stream ----
            wb_sb = const.tile([P, H], F8)
            nc.sync.dma_start(wb_sb, Wb64[:])
            wq_sb = const.tile([P, 2, H * D], BF)
            nc.sync.dma_start(wq_sb, Wq.rearrange("(o p) m -> p o m", p=P))
            wkv_sb = const.tile([P, 2, 2 * H * D], BF)
            nc.sync.dma_start(wkv_sb, Wkv.rearrange("(o p) m -> p o m", p=P))
            xqT_sb = const.tile([P, 2, NQC], BF)
            nc.sync.dma_start(xqT_sb, xqT.rearrange("(o p) q -> p o q", p=P))
            xkvT_sb = const.tile([P, 2, NK], BF)
            nc.sync.dma_start(xkvT_sb, xkvT.rearrange("(o p) k -> p o k", p=P))
            bq2_sb = const.tile([D, H], FP)
            nc.sync.dma_start(bq2_sb, bq2[:])
            bkvK2_sb = const.tile([D, H], FP)
            nc.sync.dma_start(bkvK2_sb, bkvK2[:])
            bkvV_sb = const.tile([1, H * D], BF)
            nc.sync.dma_start(bkvV_sb, bkvV[None, :])
            bbpat_sb = const.tile([1, H * QH], BF)
            nc.sync.dma_start(bbpat_sb, bbpat[None, :])
            wp_sb = const.tile([P, 2, CQ], FP)
            nc.sync.dma_start(wp_sb, Wp.rearrange("(o p) m -> p o m", p=P))
            bp_sb = const.tile([1, CQ], FP)
            nc.sync.dma_start(bp_sb, bp[None, :])

            # z stream head-start: prefetch the first 3 k-chunks (6 halves)
            z_tiles = {}
            for gidx in range(6):
                z_sb = zpool.tile([P, QH, P], F8, tag="z", name=f"zpre{gidx}")
                nc.sync.dma_start(z_sb, zT[gidx])
                z_tiles[gidx] = z_sb

            ident = const.tile([P, P], FP)
            make_identity(nc, ident)
            ones_bf = const.tile([1, P], BF)
            nc.vector.memset(ones_bf, 1.0)

            # V augmented with a ones column per head: [k, kc, h, D+1]
            vaug_sb = const.tile([P, KC_N, H, D + 1], BF)
            nc.vector.memset(vaug_sb, 1.0)

            # ---- projections (bf16 in, fp32 psum accumulate) ----
            # per-head Q^T/K^T on partitions 0-31: [32, h, n]
            # Q^T with (x + bq) * 64*SCALE folded in, stored bf16
            qT_sb = const.tile([D, H, NQC], BF)
            kT_sb = const.tile([D, H, NK], BF)
            for h in range(H):
                ps = proj_ps.tile([P, 512], FP, tag="proj", name="q_ps")
                for c in range(2):
                    nc.tensor.matmul(ps[0:D, 0:NQC],
                                     lhsT=wq_sb[:, c, h * D:(h + 1) * D],
                                     rhs=xqT_sb[:, c, :],
                                     start=(c == 0), stop=(c == 1))
                nc.vector.tensor_scalar(qT_sb[:, h, :], ps[0:D, 0:NQC],
                                        bq2_sb[:, h:h + 1], WBS * SCALE,
                                        add, mult)
            for h in range(H):
                for nh in range(NK // 512):
                    ps = proj_ps.tile([P, 512], FP, tag="proj", name="k_ps")
                    for c in range(2):
                        nc.tensor.matmul(ps[0:D, :],
                                         lhsT=wkv_sb[:, c, h * D:(h + 1) * D],
                                         rhs=xkvT_sb[:, c,
                                                     nh * 512:(nh + 1) * 512],
                                         start=(c == 0), stop=(c == 1))
                    nc.vector.tensor_scalar(kT_sb[:, h, nh * 512:(nh + 1) * 512],
                                            ps[0:D, :], bkvK2_sb[:, h:h + 1],
                                            None, add)

            # V [k, (h d)] + bkv_V, written into vaug (ones col preserved)
            for kc in range(KC_N):
                ps = proj_ps.tile([P, 512], FP, tag="proj", name="v_ps")
                for c in range(2):
                    nc.tensor.matmul(ps[:, :H * D],
                                     lhsT=xkvT_sb[:, c, kc * P:(kc + 1) * P],
                                     rhs=wkv_sb[:, c, H * D:2 * H * D],
                                     start=(c == 0), stop=False)
                nc.tensor.matmul(ps[:, :H * D], lhsT=ones_bf,
                                 rhs=bkvV_sb, start=False, stop=True)
                nc.scalar.activation(
                    vaug_sb[:, kc, :, 0:D],
                    ps[:, :H * D].rearrange("p (h d) -> p h d", h=H),
                    mybir.ActivationFunctionType.Copy)

            # ---- main loop over k-chunks ----
            o_ps = o_psp.tile([P, H * (D + 1)], FP)
            prev_e = None
            for kc in range(KC_N):
                # S^T banks [k, (4h, 128q)], strip-0 matmuls only
                s_tiles = []
                for hg in range(2):
                    s_ps = s_psp.tile([P, 4, NQC], FP, tag="s")
                    for hl in range(4):
                        h = hg * 4 + hl
                        nc.tensor.matmul(
                            s_ps[:, hl, :],
                            lhsT=kT_sb[:, h, kc * P:(kc + 1) * P],
                            rhs=qT_sb[:, h, :],
                            start=(hl == 0), stop=(hl == 3))
                    s_tiles.append(s_ps)

                # bias banks [k, (64q, 8h)]: ones x bb + 64 z matmuls
                b_tiles = []
                for hf in range(2):
                    gidx = kc * 2 + hf
                    nxt = gidx + 6
                    if nxt < 2 * KC_N:
                        zn = zpool.tile([P, QH, P], F8, tag="z")
                        nc.sync.dma_start(zn, zT[nxt])
                        z_tiles[nxt] = zn
                    z_sb = z_tiles.pop(gidx)
                    b_ps = b_psp.tile([P, QH, H], FP, tag="b")
                    nc.tensor.matmul(b_ps.rearrange("p q h -> p (q h)"),
                                     lhsT=ones_bf, rhs=bbpat_sb,
                                     start=True, stop=False)
                    for qi in range(QH):
                        nc.tensor.matmul(b_ps[:, qi, :],
                                         lhsT=z_sb[:, qi, :], rhs=wb_sb,
                                         start=False, stop=(qi == QH - 1))
                    b_tiles.append(b_ps)

                    # AV for the previous k-chunk between the two bias banks
                    if hf == 0 and prev_e is not None:
                        pkc, pe = prev_e
                        for h in range(H):
                            nc.tensor.matmul(
                                o_ps[:, h * (D + 1):(h + 1) * (D + 1)],
                                lhsT=pe[h // 4][:, h % 4, :],
                                rhs=vaug_sb[:, pkc, h, :],
                                start=(pkc == 0 and h == 0), stop=False)

                # exp((S'+bias'+bb')/64) = exp(S'/64) * exp((bias'+bb')/64)
                es, eb, e_sb = [], [], []
                for hg in range(2):
                    t = epool.tile([P, 4, NQC], FP, tag=f"es{hg}",
                                   name=f"es{hg}")
                    nc.scalar.activation(t, s_tiles[hg],
                                         mybir.ActivationFunctionType.Exp,
                                         scale=1.0 / WBS)
                    es.append(t)
                for hf in range(2):
                    t = epool.tile([P, QH, H], FP, tag=f"eb{hf}",
                                   name=f"eb{hf}")
                    nc.scalar.activation(t, b_tiles[hf],
                                         mybir.ActivationFunctionType.Exp,
                                         scale=1.0 / WBS)
                    eb.append(t)
                for hg in range(2):
                    t = epool.tile([P, 4, NQC], BF, tag=f"e{hg}",
                                   name=f"e{hg}")
                    e_sb.append(t)
                for hg in range(2):
                    for hf in range(2):
                        nc.vector.tensor_tensor(
                            e_sb[hg][:, :, hf * QH:(hf + 1) * QH],
                            es[hg][:, :, hf * QH:(hf + 1) * QH],
                            eb[hf][:, :, hg * 4:(hg + 1) * 4]
                               .rearrange("p q h -> p h q"),
                            mult)
                # single whole-tile rewrite: the AV weight loads must wait on
                # ONE writer covering the full tile (two half-writers race
                # with the PE LDWEIGHTS pull-ahead on hardware)
                e2_sb = []
                for hg in range(2):
                    t = epool.tile([P, 4, NQC], BF, tag=f"e2{hg}",
                                   name=f"e2{hg}")
                    nc.vector.tensor_copy(t, e_sb[hg])
                    e2_sb.append(t)
                prev_e = (kc, e2_sb)

            # final AV chunk
            pkc, pe = prev_e
            for h in range(H):
                nc.tensor.matmul(o_ps[:, h * (D + 1):(h + 1) * (D + 1)],
                                 lhsT=pe[h // 4][:, h % 4, :],
                                 rhs=vaug_sb[:, pkc, h, :],
                                 start=False, stop=(h == H - 1))

            # ---- epilogue: normalize, transpose, output projection ----
            recip_sb = const.tile([P, H], FP)
            for h in range(H):
                nc.vector.reciprocal(recip_sb[:, h:h + 1],
                                     o_ps[:, h * (D + 1) + D:h * (D + 1) + D + 1])
            o_sb = const.tile([P, 2, P], FP)     # [q, half, (h d)%128]
            for h in range(H):
                nc.vector.tensor_scalar(
                    o_sb[:, h // 4, (h % 4) * 32:(h % 4) * 32 + 32],
                    o_ps[:, h * (D + 1):h * (D + 1) + D],
                    recip_sb[:, h:h + 1], None, mult)
            oT_sb = const.tile([P, 2, P], FP)
            for m in range(2):
                t_full = proj_ps.tile([P, 512], FP, tag="proj", name="t_full")
                t_ps = t_full[:, :P]
                nc.tensor.transpose(t_ps, o_sb[:, m, :], ident)
                nc.vector.tensor_copy(oT_sb[:, m, :], t_ps)
            ones_fp = const.tile([1, P], FP)
            nc.vector.memset(ones_fp, 1.0)
            ps = proj_ps.tile([P, 512], FP, tag="proj")
            for m in range(2):
                nc.tensor.matmul(ps[:, :CQ], lhsT=oT_sb[:, m, :],
                                 rhs=wp_sb[:, m, :], start=(m == 0), stop=False)
            nc.tensor.matmul(ps[:, :CQ], lhsT=ones_fp, rhs=bp_sb,
                             start=False, stop=True)
            y_sb = const.tile([P, CQ], FP)
            nc.vector.tensor_copy(y_sb, ps[:, :CQ])
            nc.sync.dma_start(y[:], y_sb)

    nc.compile()
    return nc


def prep_inputs(x_q, x_kv, z, Wq, bq, Wkv, bkv, Wb, bb, Wp, bp):
    """Host-side shard prep.  Returns in_maps for the 8 cores."""
    xkvT = np.ascontiguousarray(x_kv[0].T).astype(NP_BF)     # [CKV, nk]
    shared = dict(xkvT=xkvT,
                  Wq=np.ascontiguousarray(Wq).astype(NP_BF),
                  bq2=np.ascontiguousarray(
                      np.asarray(bq).reshape(H, D).T, dtype=np.float32),
                  Wkv=np.ascontiguousarray(Wkv).astype(NP_BF),
                  bkvK2=np.ascontiguousarray(
                      np.asarray(bkv)[:H * D].reshape(H, D).T,
                      dtype=np.float32),
                  bkvV=np.ascontiguousarray(bkv[H * D:]).astype(NP_BF),
                  Wb64=np.ascontiguousarray(Wb * WBS).astype(NP_F8),
                  bbpat=np.tile(np.asarray(bb) * WBS, QH).astype(NP_BF),
                  Wp=np.ascontiguousarray(Wp, dtype=np.float32),
                  bp=np.ascontiguousarray(bp, dtype=np.float32))
    in_maps = []
    for i in range(NCORES):
        qs = i * NQC
        zi = z[0, qs:qs + NQC]                   # [q, k, c]
        # -> [kc*2+half, c, q64, k128]
        zi = zi.reshape(2, QH, KC_N, P, BD).transpose(2, 0, 4, 1, 3)
        zi = zi.reshape(2 * KC_N, BD, QH, P)
        in_maps.append(dict(
            zT=np.ascontiguousarray(zi).astype(NP_F8),
            xqT=np.ascontiguousarray(x_q[0, qs:qs + NQC].T).astype(NP_BF),
            **shared,
        ))
    return in_maps


_NC_CACHE = {}


def kernel(x_q, x_kv, z, Wq, bq, Wkv, bkv, Wb, bb, Wp, bp):
    key = "full"
    if key not in _NC_CACHE:
        _NC_CACHE[key] = build_program()
    nc = _NC_CACHE[key]
    in_maps = prep_inputs(x_q, x_kv, z, Wq, bq, Wkv, bkv, Wb, bb, Wp, bp)
    res = run_bass_kernel_spmd(nc, in_maps, list(range(NCORES)))
    out = np.empty((1, NQ, CQ), dtype=np.float32)
    for i in range(NCORES):
        out[0, i * NQC:(i + 1) * NQC, :] = res.results[i]["y"]
    return out


# revision 20
# speedup vs baseline: 1.3080x; 1.1497x over previous
"""BiasAttention TRN2 kernel — q-sharded across 8 NeuronCores, fp8 z.

Each core owns 128 queries and computes full attention for them (8 heads,
1024 keys) with no collectives.  The dominant cost is streaming the bias
tensor z ([q,k,c] = 16.8 MB/core in fp8): host prep casts z to e3m4
(4 mantissa bits; z~N(0,1) fits the +-15.5 range) and lays it out
[kc, c, q, k] so each [c=128, k=128] slice is LDWEIGHTS-ready; fp8
weight loads stream at ~27 ns/tile (4 B/cycle fast path).  One 2 MB DMA
per k-chunk, alternating between the two HWDGE rings (sync/scalar), keeps
the stream off the per-transfer latency floor.

Scores are computed directly in the transposed frame S^T[k, q] (lhsT = K^T)
so the exp output feeds the AV matmul without any PE transposes.  Q^T/K^T
are stored per-head on partitions 0-31 ([32, h, n]) so score matmuls never
need tile_position (multi-matmul PSUM accumulation with row-positioned
strips aborts at load on this runtime).  Per k-chunk: two score banks
[k, (4h, 128q)] and two bias banks [k, (64q, 8h)] accumulating a
ones x bb row plus 64 z matmuls (rhs = e3m4(64*Wb)).  The 1/64 Wb scale
(needed to lift Wb out of e3m4's subnormal range) is folded into the exp
activations via exp(a+b) = exp(a)*exp(b): each PSUM bank gets its own exp
(ACT reads one PSUM operand) and a DVE multiply fuses them into bf16
slabs that are the AV lhsT as-is; a single whole-tile rewrite then feeds
the AV weight loads (two half-writers race with the PE LDWEIGHTS
pull-ahead on hardware).  AV for chunk kc runs during chunk kc+2 so the
ACT/DVE chain never stalls the PE.  A ones column appended to V
accumulates the softmax denominator in the same AV pass.
"""

import sys

if "/opt/trn_rl_repo" not in sys.path:
    sys.path.insert(0, "/opt/trn_rl_repo")

import ml_dtypes
import numpy as np

import concourse.bass as bass
import concourse.mybir as mybir
from concourse import bacc
from concourse.bass_utils import run_bass_kernel_spmd
from concourse.masks import make_identity
from concourse.tile import TileContext

P = 128          # partitions
H = 8            # heads
D = 32           # head dim
CQ = 256         # q channels
CKV = 256        # kv channels
BD = 128         # bias (z) channels
NQ = 1024        # total queries
NK = 1024        # total keys
NCORES = 8
NQC = NQ // NCORES   # queries per core = 128
KC_N = NK // P       # k-chunks of 128
QH = NQC // 2        # queries per bias psum bank = 64
SCALE = D ** (-0.5)
WBS = 64.0           # Wb host-side scale (power of two)

FP = mybir.dt.float32
BF = mybir.dt.bfloat16
F8 = mybir.dt.float8e3
NP_BF = ml_dtypes.bfloat16
NP_F8 = ml_dtypes.float8_e3m4


def build_program():
    add = mybir.AluOpType.add
    mult = mybir.AluOpType.mult

    nc = bacc.Bacc("TRN2", target_bir_lowering=False, debug=False,
                   num_devices=NCORES)

    # ---- I/O ----
    zT = nc.dram_tensor("zT", [KC_N, BD, NQC, P], F8, kind="ExternalInput")
    xqT = nc.dram_tensor("xqT", [CQ, NQC], BF, kind="ExternalInput")
    xkvT = nc.dram_tensor("xkvT", [CKV, NK], BF, kind="ExternalInput")
    Wq = nc.dram_tensor("Wq", [CQ, H * D], BF, kind="ExternalInput")
    bq2 = nc.dram_tensor("bq2", [D, H], FP, kind="ExternalInput")
    Wkv = nc.dram_tensor("Wkv", [CKV, 2 * H * D], BF, kind="ExternalInput")
    bkvK2 = nc.dram_tensor("bkvK2", [D, H], FP, kind="ExternalInput")
    bkvV = nc.dram_tensor("bkvV", [H * D], BF, kind="ExternalInput")
    Wb64 = nc.dram_tensor("Wb64", [BD, H], F8, kind="ExternalInput")
    bbpat = nc.dram_tensor("bbpat", [H * QH], BF, kind="ExternalInput")
    Wp = nc.dram_tensor("Wp", [H * D, CQ], FP, kind="ExternalInput")
    bp = nc.dram_tensor("bp", [CQ], FP, kind="ExternalInput")
    y = nc.dram_tensor("y", [NQC, CQ], FP, kind="ExternalOutput")

    with TileContext(nc) as tc:
        with (
            tc.tile_pool(name="const", bufs=1) as const,
            tc.tile_pool(name="zpool", bufs=4) as zpool,
            tc.tile_pool(name="epool", bufs=3) as epool,
            tc.tile_pool(name="proj_ps", bufs=2, space="PSUM") as proj_ps,
            tc.tile_pool(name="b_ps", bufs=3, space="PSUM") as b_psp,
            tc.tile_pool(name="s_ps", bufs=2, space="PSUM") as s_psp,
            tc.tile_pool(name="o_ps", bufs=1, space="PSUM") as o_psp,
        ):
            # ---- DMA order: Q/K/V inputs first (prologue deps), then the
            # z stream on both HWDGE rings, then late-use weights ----
            wq_sb = const.tile([P, 2, H * D], BF)
            nc.sync.dma_start(wq_sb, Wq.rearrange("(o p) m -> p o m", p=P))
            xqT_sb = const.tile([P, 2, NQC], BF)
            nc.sync.dma_start(xqT_sb, xqT.rearrange("(o p) q -> p o q", p=P))
            wkv_sb = const.tile([P, 2, 2 * H * D], BF)
            nc.sync.dma_start(wkv_sb, Wkv.rearrange("(o p) m -> p o m", p=P))
            xkvT_sb = const.tile([P, 2, NK], BF)
            nc.sync.dma_start(xkvT_sb, xkvT.rearrange("(o p) k -> p o k", p=P))
            bq2_sb = const.tile([D, H], FP)
            nc.scalar.dma_start(bq2_sb, bq2[:])
            bkvK2_sb = const.tile([D, H], FP)
            nc.scalar.dma_start(bkvK2_sb, bkvK2[:])
            bkvV_sb = const.tile([1, H * D], BF)
            nc.scalar.dma_start(bkvV_sb, bkvV[None, :])
            wb_sb = const.tile([P, H], F8)
            nc.scalar.dma_start(wb_sb, Wb64[:])
            bbpat_sb = const.tile([1, H * QH], BF)
            nc.scalar.dma_start(bbpat_sb, bbpat[None, :])

            # z stream head-start: prefetch first 3 chunks across the rings
            z_tiles = {}
            for gidx in range(3):
                z_sb = zpool.tile([P, NQC, P], F8, tag="z", name=f"zpre{gidx}")
                eng = nc.sync if gidx % 2 == 0 else nc.scalar
                eng.dma_start(z_sb, zT[gidx])
                z_tiles[gidx] = z_sb

            wp_sb = const.tile([P, 2, CQ], FP)
            nc.scalar.dma_start(wp_sb, Wp.rearrange("(o p) m -> p o m", p=P))
            bp_sb = const.tile([1, CQ], FP)
            nc.scalar.dma_start(bp_sb, bp[None, :])

            ident = const.tile([P, P], FP)
            make_identity(nc, ident)
            ones_bf = const.tile([1, P], BF)
            nc.vector.memset(ones_bf, 1.0)

            # PE warmup while the first DMAs land: un-throttle the HAM clock
            # gate with identity matmuls so the real work runs at 2.4 GHz
            warm_ps = proj_ps.tile([P, 512], FP, tag="proj", name="warm_ps")
            for w in range(8):
                nc.tensor.matmul(warm_ps[:, :P], lhsT=ident, rhs=ident,
                                 start=(w == 0), stop=(w == 7))

            # V augmented with a ones column per head: [k, kc, h, D+1]
            vaug_sb = const.tile([P, KC_N, H, D + 1], BF)
            nc.vector.memset(vaug_sb, 1.0)

            # ---- projections (bf16 in, fp32 psum accumulate) ----
            # per-head Q^T/K^T on partitions 0-31: [32, h, n]
            # Q^T with (x + bq) * 64*SCALE folded in, stored bf16
            qT_sb = const.tile([D, H, NQC], BF)
            kT_sb = const.tile([D, H, NK], BF)
            for h in range(H):
                ps = proj_ps.tile([P, 512], FP, tag="proj", name="q_ps")
                for c in range(2):
                    nc.tensor.matmul(ps[0:D, 0:NQC],
                                     lhsT=wq_sb[:, c, h * D:(h + 1) * D],
                                     rhs=xqT_sb[:, c, :],
                                     start=(c == 0), stop=(c == 1))
                nc.vector.tensor_scalar(qT_sb[:, h, :], ps[0:D, 0:NQC],
                                        bq2_sb[:, h:h + 1], WBS * SCALE,
                                        add, mult)
            for h in range(H):
                for nh in range(NK // 512):
                    ps = proj_ps.tile([P, 512], FP, tag="proj", name="k_ps")
                    for c in range(2):
                        nc.tensor.matmul(ps[0:D, :],
                                         lhsT=wkv_sb[:, c, h * D:(h + 1) * D],
                                         rhs=xkvT_sb[:, c,
                                                     nh * 512:(nh + 1) * 512],
                                         start=(c == 0), stop=(c == 1))
                    nc.vector.tensor_scalar(kT_sb[:, h, nh * 512:(nh + 1) * 512],
                                            ps[0:D, :], bkvK2_sb[:, h:h + 1],
                                            None, add)

            # V [k, (h d)] + bkv_V, written into vaug (ones col preserved)
            for kc in range(KC_N):
                ps = proj_ps.tile([P, 512], FP, tag="proj", name="v_ps")
                for c in range(2):
                    nc.tensor.matmul(ps[:, :H * D],
                                     lhsT=xkvT_sb[:, c, kc * P:(kc + 1) * P],
                                     rhs=wkv_sb[:, c, H * D:2 * H * D],
                                     start=(c == 0), stop=False)
                nc.tensor.matmul(ps[:, :H * D], lhsT=ones_bf,
                                 rhs=bkvV_sb, start=False, stop=True)
                nc.scalar.activation(
                    vaug_sb[:, kc, :, 0:D],
                    ps[:, :H * D].rearrange("p (h d) -> p h d", h=H),
                    mybir.ActivationFunctionType.Copy)

            # ---- main loop over k-chunks ----
            o_ps = o_psp.tile([P, H * (D + 1)], FP)
            e_hist = []    # [(kc, [e2_hg0, e2_hg1])], AV lags 2 chunks

            def emit_av(pkc, pe, first, last):
                for h in range(H):
                    nc.tensor.matmul(
                        o_ps[:, h * (D + 1):(h + 1) * (D + 1)],
                        lhsT=pe[h // 4][:, h % 4, :],
                        rhs=vaug_sb[:, pkc, h, :],
                        start=(first and h == 0),
                        stop=(last and h == H - 1))

            for kc in range(KC_N):
                nxt = kc + 3
                if nxt < KC_N:
                    zn = zpool.tile([P, NQC, P], F8, tag="z")
                    eng = nc.sync if nxt % 2 == 0 else nc.scalar
                    eng.dma_start(zn, zT[nxt])
                    z_tiles[nxt] = zn
                z_sb = z_tiles.pop(kc)

                # S^T banks [k, (4h, 128q)], strip-0 matmuls only
                s_tiles = []
                for hg in range(2):
                    s_ps = s_psp.tile([P, 4, NQC], FP, tag="s")
                    for hl in range(4):
                        h = hg * 4 + hl
                        nc.tensor.matmul(
                            s_ps[:, hl, :],
                            lhsT=kT_sb[:, h, kc * P:(kc + 1) * P],
                            rhs=qT_sb[:, h, :],
                            start=(hl == 0), stop=(hl == 3))
                    s_tiles.append(s_ps)

                # bias banks [k, (64q, 8h)]: ones x bb + 64 z matmuls;
                # AV for chunk kc-2 interleaves between the two banks
                b_tiles = []
                for hf in range(2):
                    b_ps = b_psp.tile([P, QH, H], FP, tag="b")
                    nc.tensor.matmul(b_ps.rearrange("p q h -> p (q h)"),
                                     lhsT=ones_bf, rhs=bbpat_sb,
                                     start=True, stop=False)
                    for qi in range(QH):
                        nc.tensor.matmul(b_ps[:, qi, :],
                                         lhsT=z_sb[:, hf * QH + qi, :],
                                         rhs=wb_sb,
                                         start=False, stop=(qi == QH - 1))
                    b_tiles.append(b_ps)
                    if hf == 0 and len(e_hist) >= 2:
                        pkc, pe = e_hist.pop(0)
                        emit_av(pkc, pe, first=(pkc == 0), last=False)

                # exp((S'+bias'+bb')/64) = exp(S'/64) * exp((bias'+bb')/64)
                es, eb, e_sb = [], [], []
                for hg in range(2):
                    t = epool.tile([P, 4, NQC], FP, tag=f"es{hg}",
                                   name=f"es{hg}")
                    nc.scalar.activation(t, s_tiles[hg],
                                         mybir.ActivationFunctionType.Exp,
                                         scale=1.0 / WBS)
                    es.append(t)
                for hf in range(2):
                    t = epool.tile([P, QH, H], FP, tag=f"eb{hf}",
                                   name=f"eb{hf}")
                    nc.scalar.activation(t, b_tiles[hf],
                                         mybir.ActivationFunctionType.Exp,
                                         scale=1.0 / WBS)
                    eb.append(t)
                for hg in range(2):
                    t = epool.tile([P, 4, NQC], BF, tag=f"e{hg}",
                                   name=f"e{hg}")
                    e_sb.append(t)
                for hg in range(2):
                    for hf in range(2):
                        nc.vector.tensor_tensor(
                            e_sb[hg][:, :, hf * QH:(hf + 1) * QH],
                            es[hg][:, :, hf * QH:(hf + 1) * QH],
                            eb[hf][:, :, hg * 4:(hg + 1) * 4]
                               .rearrange("p q h -> p h q"),
                            mult)
                # single whole-tile rewrite: the AV weight loads must wait on
                # ONE writer covering the full tile (two half-writers race
                # with the PE LDWEIGHTS pull-ahead on hardware)
                e2_sb = []
                for hg in range(2):
                    t = epool.tile([P, 4, NQC], BF, tag=f"e2{hg}",
                                   name=f"e2{hg}")
                    nc.vector.tensor_copy(t, e_sb[hg])
                    e2_sb.append(t)
                e_hist.append((kc, e2_sb))

            # drain the AV pipeline
            while e_hist:
                pkc, pe = e_hist.pop(0)
                emit_av(pkc, pe, first=(pkc == 0), last=(pkc == KC_N - 1))

            # ---- epilogue: normalize, transpose, output projection ----
            # batched reciprocal of the 8 denominator columns (stride D+1)
            recip_sb = const.tile([P, H], FP)
            nc.vector.reciprocal(
                recip_sb, o_ps.rearrange("p (h x) -> p h x", x=D + 1)[:, :, D])
            # broadcast recip over d (stride-0 read) and fold the normalize
            # into one multiply straight out of PSUM
            recipb_ap = bass.AP(tensor=recip_sb.tensor, offset=recip_sb.offset,
                                ap=list(recip_sb.ap[:1]) + [[1, H], [0, D]])
            o_sb = const.tile([P, 2, P], FP)     # [q, half, (h d)%128]
            nc.vector.tensor_tensor(
                o_sb.rearrange("p o m -> p (o m)")
                    .rearrange("p (h d) -> p h d", h=H),
                o_ps.rearrange("p (h x) -> p h x", x=D + 1)[:, :, 0:D],
                recipb_ap, mult)
            oT_sb = const.tile([P, 2, P], FP)
            for m in range(2):
                t_full = proj_ps.tile([P, 512], FP, tag="proj", name="t_full")
                t_ps = t_full[:, :P]
                nc.tensor.transpose(t_ps, o_sb[:, m, :], ident)
                nc.vector.tensor_copy(oT_sb[:, m, :], t_ps)
            ones_fp = const.tile([1, P], FP)
            nc.vector.memset(ones_fp, 1.0)
            ps = proj_ps.tile([P, 512], FP, tag="proj")
            for m in range(2):
                nc.tensor.matmul(ps[:, :CQ], lhsT=oT_sb[:, m, :],
                                 rhs=wp_sb[:, m, :], start=(m == 0), stop=False)
            nc.tensor.matmul(ps[:, :CQ], lhsT=ones_fp, rhs=bp_sb,
                             start=False, stop=True)
            y_sb = const.tile([P, CQ], FP)
            nc.vector.tensor_copy(y_sb, ps[:, :CQ])
            nc.sync.dma_start(y[:], y_sb)

    nc.compile()
    return nc


def prep_inputs(x_q, x_kv, z, Wq, bq, Wkv, bkv, Wb, bb, Wp, bp):
    """Host-side shard prep.  Returns in_maps for the 8 cores."""
    xkvT = np.ascontiguousarray(x_kv[0].T).astype(NP_BF)     # [CKV, nk]
    shared = dict(xkvT=xkvT,
                  Wq=np.ascontiguousarray(Wq).astype(NP_BF),
                  bq2=np.ascontiguousarray(
                      np.asarray(bq).reshape(H, D).T, dtype=np.float32),
                  Wkv=np.ascontiguousarray(Wkv).astype(NP_BF),
                  bkvK2=np.ascontiguousarray(
                      np.asarray(bkv)[:H * D].reshape(H, D).T,
                      dtype=np.float32),
                  bkvV=np.ascontiguousarray(bkv[H * D:]).astype(NP_BF),
                  Wb64=np.ascontiguousarray(Wb * WBS).astype(NP_F8),
                  bbpat=np.tile(np.asarray(bb) * WBS, QH).astype(NP_BF),
                  Wp=np.ascontiguousarray(Wp, dtype=np.float32),
                  bp=np.ascontiguousarray(bp, dtype=np.float32))
    in_maps = []
    for i in range(NCORES):
        qs = i * NQC
        zi = z[0, qs:qs + NQC]                   # [q, k, c]
        # -> [kc, c, q, k128]
        zi = zi.reshape(NQC, KC_N, P, BD).transpose(1, 3, 0, 2)
        in_maps.append(dict(
            zT=np.ascontiguousarray(zi).astype(NP_F8),
            xqT=np.ascontiguousarray(x_q[0, qs:qs + NQC].T).astype(NP_BF),
            **shared,
        ))
    return in_maps


_NC_CACHE = {}


def kernel(x_q, x_kv, z, Wq, bq, Wkv, bkv, Wb, bb, Wp, bp):
    key = "full"
    if key not in _NC_CACHE:
        _NC_CACHE[key] = build_program()
    nc = _NC_CACHE[key]
    in_maps = prep_inputs(x_q, x_kv, z, Wq, bq, Wkv, bkv, Wb, bb, Wp, bp)
    res = run_bass_kernel_spmd(nc, in_maps, list(range(NCORES)))
    out = np.empty((1, NQ, CQ), dtype=np.float32)
    for i in range(NCORES):
        out[0, i * NQC:(i + 1) * NQC, :] = res.results[i]["y"]
    return out
